# revision 41
# baseline (speedup 1.0000x reference)
"""Bass/Trainium2 kernel for nn_Attn: attn = softmax_t(hidden · (W @ enc + b)).

Algebraic reorder: scores[b,t] = hidden[b] · (W @ enc[t,b] + b_attn)
                              = (hidden[b] @ W) · enc[t,b] + hidden[b]·b_attn.
The b_attn term is constant per softmax row, so it cancels in the softmax and
is dropped. We precompute v = hidden @ W (tiny PE matmul) and stream
encoder_outputs once through a fused DVE multiply+reduce — memory-bound at
one pass over the 512 MiB tensor instead of a 275 GFLOP projection.

Sharding: data-parallel over batch B=64 -> 8 NeuronCores x 8 batches.
W_attn is replicated; softmax is per-row so there is no cross-core traffic.
"""

import os
from contextlib import ExitStack

import numpy as np

import concourse.bass as bass
import concourse.tile as tile
from concourse import bacc, bass_isa, mybir
from concourse.bass_utils import run_bass_kernel_spmd

T, B, H = 2048, 64, 1024
NCORES = 8
BL = B // NCORES  # local batches per core = 8
P = 128
TCH = T // P  # t-chunks = 16
GCH = H // P  # contraction chunks for v = 8

F32 = mybir.dt.float32

# Results of the most recent run (exec_time_ns etc.), for test harnesses.
LAST_RESULTS = None


def _build_program(
    compute=True,
    softmax=True,
    prewarm=True,
    tail_split=4,
    tail_split2=8,
    body_split=8,
    dot_mode="stt",
    pipelined_softmax=True,
    h_split=True,
    per_sub_tiles=False,
    enc_bufs=3,
) -> bass.Bass:
    nc = bacc.Bacc()

    enc = nc.declare_dram_parameter("enc", [T, BL, H], F32, isOutput=False)
    # ht[p, c*BL + b] = hidden[b, c*128 + p]  (host-pretransposed layout)
    ht = nc.declare_dram_parameter("ht", [P, GCH * BL], F32, isOutput=False)
    w = nc.declare_dram_parameter("w", [H, H], F32, isOutput=False)
    # selp[k, b*128+m] = (k == b): PE broadcast helper, built host-side
    selp = nc.declare_dram_parameter("sel", [BL, BL * P], F32, isOutput=False)
    # out[p, b*TCH + c] = attn[b, c*128 + p]  (host unscrambles)
    out = nc.declare_dram_parameter("out", [P, BL * TCH], F32, isOutput=True)

    with ExitStack() as ctx:
        tc = ctx.enter_context(tile.TileContext(nc))
        singles = ctx.enter_context(tc.tile_pool(name="singles", bufs=1))
        encp = ctx.enter_context(tc.tile_pool(name="encp", bufs=enc_bufs))
        psum = ctx.enter_context(tc.tile_pool(name="psum", bufs=2, space="PSUM"))

        # ---- load W (natural [g,h] layout: g on partitions) and hiddenT
        # setup loads ride the SWDGE (gpsimd) + scalar-HWDGE queues so the
        # sync queue is dedicated to the 64 MB encoder stream, and W halves
        # land in parallel (v sits on the startup critical path)
        ht_sb = singles.tile([P, GCH * BL], F32)
        nc.gpsimd.dma_start(out=ht_sb, in_=ht[:, :])
        w_sb = singles.tile([P, GCH * H], F32)  # w_sb[p, c*H + h] = W[c*128+p, h]
        for c in range(GCH):
            eng = nc.gpsimd if c % 2 == 0 else nc.scalar
            eng.dma_start(out=w_sb[:, c * H : (c + 1) * H],
                          in_=w[c * P : (c + 1) * P, :])

        # ---- v[b,h] = sum_g hidden[b,g] W[g,h], accumulated over GCH chunks
        v_sb = singles.tile([BL, H], F32)
        for nh in range(2):  # PSUM bank free-dim limit: 512 f32
            vp = psum.tile([BL, 512], F32)
            for c in range(GCH):
                nc.tensor.matmul(
                    vp,
                    lhsT=ht_sb[:, c * BL : (c + 1) * BL],
                    rhs=w_sb[:, c * H + nh * 512 : c * H + nh * 512 + 512],
                    start=(c == 0),
                    stop=(c == GCH - 1),
                )
            if nh == 0:
                nc.vector.tensor_copy(v_sb[:, nh * 512 : (nh + 1) * 512], vp)
            else:
                nc.scalar.copy(v_sb[:, nh * 512 : (nh + 1) * 512], vp)

        # ---- broadcast each v row across all 128 partitions via PE:
        # (sel_b).T @ v_sb with sel_b[k, m] = (k == b) gives v[b, :] on every
        # partition. (gpsimd.partition_broadcast needs partition-0 sources.)
        sel = singles.tile([BL, BL * P], F32)
        nc.gpsimd.dma_start(out=sel, in_=selp[:, :])
        v_bc = singles.tile([P, BL * H], F32)  # v_bc[p, b*H + h] = v[b, h]
        for b in range(BL):
            for nh in range(2):
                bp = psum.tile([P, 512], F32)
                nc.tensor.matmul(
                    bp,
                    lhsT=sel[:, b * P : (b + 1) * P],
                    rhs=v_sb[:, nh * 512 : (nh + 1) * 512],
                    start=True,
                    stop=True,
                )
                if (b * 2 + nh) % 2 == 0:
                    nc.vector.tensor_copy(
                        v_bc[:, b * H + nh * 512 : b * H + nh * 512 + 512], bp
                    )
                else:
                    nc.scalar.copy(
                        v_bc[:, b * H + nh * 512 : b * H + nh * 512 + 512], bp
                    )

        # ---- main stream: scores[p, b*TCH+c] = sum_h enc[c*128+p, b, h] v[b, h]
        scratch = ctx.enter_context(tc.tile_pool(name="scratch", bufs=2))
        scores = singles.tile([P, BL * TCH], F32)
        dummy = singles.tile([P, 1], F32)
        if prewarm:
            # warm the Exp activation table off the critical tail
            nc.scalar.activation(
                dummy, dummy, mybir.ActivationFunctionType.Exp, bias=0.0, scale=0.0
            )
        def emit_dot(enc_ap, v_ap, accum_col):
            if dot_mode == "stt":
                # fused: out=(enc*1)*v, accum=sum(out) -> one DVE pass
                prod = scratch.tile(
                    [P, enc_ap.shape[-1]], F32, tag="prod", name="prod"
                )
                nc.vector.scalar_tensor_tensor(
                    out=prod,
                    in0=enc_ap,
                    scalar=1.0,
                    in1=v_ap,
                    op0=mybir.AluOpType.mult,
                    op1=mybir.AluOpType.mult,
                    accum_out=accum_col,
                )
            else:  # "act": DVE multiplies, ACT reduces (copy with accum_out)
                prod = scratch.tile(
                    [P, enc_ap.shape[-1]], F32, tag="prod", name="prod"
                )
                nc.vector.tensor_mul(prod, enc_ap, v_ap)
                sink = scratch.tile(
                    [P, enc_ap.shape[-1]], F32, tag="sink", name="sink"
                )
                nc.scalar.activation(
                    sink,
                    prod,
                    mybir.ActivationFunctionType.Copy,
                    bias=0.0,
                    scale=1.0,
                    accum_out=accum_col,
                )

        for c in range(TCH):
            # split the trailing tiles' DMA+compute finer so the last DVE
            # work pipelines behind the last bytes instead of lagging 10us
            if c == TCH - 1:
                nsub = tail_split2
            elif c == TCH - 2:
                nsub = tail_split
            else:
                nsub = body_split
            enc_t = None if per_sub_tiles else encp.tile([P, BL, H], F32)
            bl_sub = BL // nsub
            if c == TCH - 1 and h_split and nsub == BL and compute:
                # final tile: per-b AND per-h-half splits so the very last
                # dot is a 512-wide op lagging the last byte by ~0.7us;
                # halves merge via tensor_scalar_add
                HH = H // 2
                for b in range(BL):
                    if per_sub_tiles:
                        enc_t = encp.tile([P, 1, H], F32, tag="enc_s", name="enc_s")
                        bb = 0
                    else:
                        bb = b
                    halves = scratch.tile([P, 2], F32, tag="hmerge", name="halves")
                    for hh in range(2):
                        nc.sync.dma_start(
                            out=enc_t[:, bb : bb + 1, hh * HH : (hh + 1) * HH],
                            in_=enc[
                                c * P : (c + 1) * P, b : b + 1, hh * HH : (hh + 1) * HH
                            ],
                        )
                        emit_dot(
                            enc_t[:, bb, hh * HH : (hh + 1) * HH],
                            v_bc[:, b * H + hh * HH : b * H + (hh + 1) * HH],
                            halves[:, hh : hh + 1],
                        )
                    nc.vector.tensor_scalar_add(
                        scores[:, b * TCH + c : b * TCH + c + 1],
                        halves[:, 0:1],
                        halves[:, 1:2],
                    )
                continue
            for s in range(nsub):
                if per_sub_tiles:
                    enc_t = encp.tile(
                        [P, bl_sub, H], F32, tag="enc_s", name="enc_s"
                    )
                    boff = s * bl_sub
                else:
                    boff = 0
                nc.sync.dma_start(
                    out=enc_t[:, s * bl_sub - boff : (s + 1) * bl_sub - boff, :],
                    in_=enc[c * P : (c + 1) * P, s * bl_sub : (s + 1) * bl_sub, :],
                )
                if not compute:
                    continue
                for b in range(s * bl_sub, (s + 1) * bl_sub):
                    emit_dot(
                        enc_t[:, b - boff, :],
                        v_bc[:, b * H : (b + 1) * H],
                        scores[:, b * TCH + c : b * TCH + c + 1],
                    )

        # ---- softmax over t (spread across partitions p x chunks c) per b
        if not softmax or not compute:
            nc.sync.dma_start(out=out[:, :], in_=scores)
            nc.finalize()
            return nc
        rowmax = singles.tile([P, BL], F32)
        gmax = singles.tile([P, BL], F32)
        negmax = singles.tile([P, BL], F32)
        probs = singles.tile([P, BL * TCH], F32)
        rowsum = singles.tile([P, BL], F32)
        gsum = singles.tile([P, BL], F32)
        rsum = singles.tile([P, BL], F32)
        if pipelined_softmax:
            # one independent chain per b: each starts as soon as that b's
            # scores complete (last-tile subs arrive b-by-b), so only the
            # final b's chain trails the last DMA byte
            for b in range(BL):
                bl, bh = b * TCH, (b + 1) * TCH
                nc.vector.reduce_max(
                    rowmax[:, b : b + 1], scores[:, bl:bh],
                    axis=mybir.AxisListType.X,
                )
                nc.gpsimd.partition_all_reduce(
                    gmax[:, b : b + 1], rowmax[:, b : b + 1], P,
                    bass_isa.ReduceOp.max,
                )
                # negate on ACT: it feeds ACT's exp next, saving a hop via DVE
                nc.scalar.mul(negmax[:, b : b + 1], gmax[:, b : b + 1], -1.0)
                nc.scalar.activation(
                    probs[:, bl:bh], scores[:, bl:bh],
                    mybir.ActivationFunctionType.Exp,
                    bias=negmax[:, b : b + 1], scale=1.0,
                    accum_out=rowsum[:, b : b + 1],
                )
                nc.gpsimd.partition_all_reduce(
                    gsum[:, b : b + 1], rowsum[:, b : b + 1], P,
                    bass_isa.ReduceOp.add,
                )
                nc.vector.reciprocal(rsum[:, b : b + 1], gsum[:, b : b + 1])
                nc.vector.tensor_scalar_mul(
                    probs[:, bl:bh], probs[:, bl:bh], rsum[:, b : b + 1]
                )
        else:
            for b in range(BL):
                nc.vector.reduce_max(
                    rowmax[:, b : b + 1],
                    scores[:, b * TCH : (b + 1) * TCH],
                    axis=mybir.AxisListType.X,
                )
            nc.gpsimd.partition_all_reduce(gmax, rowmax, P, bass_isa.ReduceOp.max)
            nc.vector.tensor_scalar_mul(negmax, gmax, -1.0)
            for b in range(BL):
                nc.scalar.activation(
                    probs[:, b * TCH : (b + 1) * TCH],
                    scores[:, b * TCH : (b + 1) * TCH],
                    mybir.ActivationFunctionType.Exp,
                    bias=negmax[:, b : b + 1],
                    scale=1.0,
                    accum_out=rowsum[:, b : b + 1],
                )
            nc.gpsimd.partition_all_reduce(gsum, rowsum, P, bass_isa.ReduceOp.add)
            nc.vector.reciprocal(rsum, gsum)
            for b in range(BL):
                nc.vector.tensor_scalar_mul(
                    probs[:, b * TCH : (b + 1) * TCH],
                    probs[:, b * TCH : (b + 1) * TCH],
                    rsum[:, b : b + 1],
                )

        nc.sync.dma_start(out=out[:, :], in_=probs)

    nc.finalize()
    return nc


_PROGRAM = None


def _program() -> bass.Bass:
    global _PROGRAM
    if _PROGRAM is None:
        _PROGRAM = _build_program()
    return _PROGRAM


SEL = np.kron(np.eye(BL, dtype=np.float32), np.ones((1, P), dtype=np.float32))


def make_in_maps(hidden, encoder_outputs, W_attn):
    """Shard inputs for the 8 cores. hidden [1,B,H], enc [T,B,H], W [H,H]."""
    in_maps = []
    w = np.ascontiguousarray(W_attn, dtype=np.float32)
    for i in range(NCORES):
        b0 = i * BL
        enc_shard = np.ascontiguousarray(encoder_outputs[:, b0 : b0 + BL, :],
                                         dtype=np.float32)
        h = np.asarray(hidden[0, b0 : b0 + BL, :], dtype=np.float32)  # [BL, H]
        # ht[p, c*BL+b] = h[b, c*128+p]
        ht = np.ascontiguousarray(
            h.T.reshape(GCH, P, BL).transpose(1, 0, 2).reshape(P, GCH * BL)
        )
        in_maps.append({"enc": enc_shard, "ht": ht, "w": w, "sel": SEL})
    return in_maps


def unshard_output(results):
    """results[i]["out"] is [128, BL*TCH]; reassemble to [B, 1, T] float32."""
    full = np.empty((B, 1, T), dtype=np.float32)
    for i, res in enumerate(results):
        arr = np.asarray(res["out"])  # [P, BL*TCH]
        blk = arr.reshape(P, BL, TCH).transpose(1, 2, 0).reshape(BL, T)
        full[i * BL : (i + 1) * BL, 0, :] = blk
    return full


def kernel(hidden, encoder_outputs, W_attn, b_attn):
    """Full inputs in, full output out. b_attn is provably irrelevant (softmax
    shift invariance); asserting nothing about it beyond shape."""
    global LAST_RESULTS
    nc = _program()
    in_maps = make_in_maps(hidden, encoder_outputs, W_attn)
    trace = os.environ.get("BASS_KERNEL_TRACE") == "1"
    res = run_bass_kernel_spmd(nc, in_maps, list(range(NCORES)), trace=trace)
    LAST_RESULTS = res
    return unshard_output(res.results)
